# revision 11
# baseline (speedup 1.0000x reference)
"""BitLinear-1.58 (absmean ternary quant + linear) on 8 TRN2 NeuronCores.

Problem: x[4, 2048, 4096] f32, weight[16384, 4096] f32, bias[16384] f32.
    w_q = sign(w) * (|w| >= 0.7 * mean(|w|))   (global mean over all of w)
    y   = x @ w_q.T + bias                      -> [4, 2048, 16384] f32

Sharding (column/tensor parallel): weight & bias sharded along
out_features across 8 cores (2048 each); x replicated. Each core
computes y_shard [8192, 2048]; the host concatenates shards.

Matmul runs in fp8e4 with perf_mode=DoubleRow (256-contraction MMs).
Ternary w_q is exact in fp8. x is split on the host into x_hi = fp8(x)
and x_lo = fp8(x - x_hi); the chain contracts x_hi over all 16 k-pairs
plus x_lo over the first N_LO=8 pairs (error compensation, rel-err
1.89e-2 vs the 2e-2 gate). 24 MMs per chain, 6144 total; the PE runs
them back-to-back at ~263ns (the firmware clamps the PE to 13/16 =
1.95GHz whenever it is active, so 512 cols / 1.95GHz is the floor).

v2 schedule (from perfetto/NTFF analysis of the v1 run @1807.8us):
  - Phase A (global absmean) was the head critical path: first MM at
    178us. wA (bf16 |w|-sum input, reshaped [8192,1024] on host) now
    has absolute priority on all 3 DMA queues, weighted-striped
    (sync:scalar:gpsimd ~ 85:72:64 GB/s) so arrival order matches the
    DVE reduce FIFO and all queues drain together; wf pool deep enough
    to keep every queue pipelined. The scalar-AllReduce plumbing sits
    right behind gpsimd's (smallest) wA share so the cc trigger fires
    as soon as the local sum lands (v1 paid 11.5us trigger delay).
  - Quant was DVE-bound at 4.0us/pair (129us span) and starved the PE
    in the strip tails (16us of gaps + a 6.8us HAM cold window). Now:
    wT arrives as [128,2slot,1024] f32 pair tiles (one DMA), the
    (w <= -thr) compare runs on GpSimd, the (w>=thr)-mneg combine on
    DVE -> ~2.6us/pair, and wT pair triggers are emitted interleaved
    at a 5-pair lead so gpsimd's own wb-gated triggers can never sit
    ahead of the quant ops that free wb slots (deadlock rule).
  - Steady state is oc-major (one 24-MM chain per 512-out chunk, then
    its epilogue) so PSUM banks free early and the last tile's y
    drains overlapped -> shorter tail (v1: 16.5us after last MM).
x is fed pre-packed (host): per token tile a [128, NJ, 2, 128] fp8
block, hi k-pairs 0..15 then lo 0..7; each tile one contiguous DMA.
"""

import numpy as np
import ml_dtypes

import concourse.bacc as bacc
import concourse.mybir as mybir
import concourse.tile as tile
import concourse.bass_utils as bass_utils

F32 = mybir.dt.float32
BF16 = mybir.dt.bfloat16
F8 = mybir.dt.float8e4
DR = mybir.MatmulPerfMode.DoubleRow
ALU = mybir.AluOpType
AX = mybir.AxisListType
E4NP = ml_dtypes.float8_e4m3

N_CORES = 8
B, S, K, O_TOTAL = 4, 2048, 4096, 16384
T = B * S                  # 8192 tokens
O = O_TOTAL // N_CORES     # 2048 out features per core
KT = K // 128              # 32 k-tiles
KP = KT // 2               # 16 k-pairs (256-contraction DoubleRow MMs)
N_LO = 8                   # k-pairs with x_lo error compensation
NJ = KP + N_LO             # 24 MMs per accumulation chain
XF = NJ * 256              # fp8 bytes/partition per token tile
N_OC = O // 512            # 4 output chunks of 512
N_QC = O // 1024           # 2 quant chunks of 1024
NT = T // 128              # 64 token tiles
STRIP = 4                  # leading token tiles, k-synchronous with quant
NWA = 32                   # wA tiles [128, 2048] bf16 (4KB lines)
QLEAD = 5                  # wT pair-trigger lead over quant (< wb bufs)
INV_N = 1.0 / (O_TOTAL * K)  # 2^-26, exact power of two

_NC_CACHE = {}


def build_nc(with_bias: bool):
    nc = bacc.Bacc("TRN2", target_bir_lowering=False, debug=False,
                   num_devices=N_CORES)
    xpk = nc.dram_tensor("xpk", [T, XF], F8, kind="ExternalInput")
    # w^T f32 split by quant chunk; pair kp of chunk qc = rows
    # 256kp..256kp+255 -> one [128, 2, 1024] slot-major DMA.
    wTq = [nc.dram_tensor(f"wT{qc}", [K, 1024], F32, kind="ExternalInput")
           for qc in range(N_QC)]
    wA = nc.dram_tensor("wA", [NWA * 128, 2048], BF16, kind="ExternalInput")
    bias = nc.dram_tensor("bias", [1, O], F32, kind="ExternalInput")
    y = nc.dram_tensor("y", [T, O], BF16, kind="ExternalOutput")

    with tile.TileContext(nc) as tc:
        with (
            tc.tile_pool(name="wf", bufs=6) as wf,
            tc.tile_pool(name="wb", bufs=7) as wb,        # f32 pair tiles
            tc.tile_pool(name="mf", bufs=3) as mf,        # quant mask pairs
            tc.tile_pool(name="wqp", bufs=KP * N_QC) as wqp,  # ternary w
            tc.tile_pool(name="xp", bufs=STRIP + 1) as xp,  # x pack staging
            tc.tile_pool(name="op", bufs=12) as op,
            tc.tile_pool(name="small", bufs=1) as small,
            tc.tile_pool(name="psum", bufs=8, space="PSUM") as psum,
            tc.tile_pool(name="dram", bufs=1, space="DRAM") as dram,
        ):
            QUEUES = [nc.sync, nc.scalar, nc.gpsimd]
            # DMA queue plan (only sync/scalar/gpsimd can start DMAs; order
            # per queue == program order):
            #   all three: wA first (weighted-striped), nothing ahead of it.
            #   sync:   wA share -> x strip t0,t2 + t4,t5 -> wT pairs g%3==0
            #           -> x steady t6..
            #   scalar: wA share -> x strip t1,t3 -> wT pairs g%3==1 ->
            #           strip y leftovers / steady epilogue copies (ACT)
            #   gpsimd: wA share (smallest) -> allreduce plumbing -> thr
            #           broadcast -> [wT pairs g%3==2 interleaved with mneg
            #           ops at QLEAD] -> y outs
            # Deadlock rule: a wb-gated wT trigger on gpsimd at pair g sits
            # after mneg(g-QLEAD); slot for g frees when pair g-7 is
            # consumed, whose mneg is at g-2 < g in gpsimd FIFO. OK.

            # ---------------- phase A: global absmean threshold ----------
            # |w| (abs applied on host, bf16, 4KB lines) streamed at full
            # 3-queue rate, DVE-reduced per tile. (A PE ones-matmul fold is
            # faster for local_sum but inflates the collectives init
            # barrier 50->115us - the barrier is PE-triggered - so the cc
            # chain, which the AllReduce queues behind, ends up later.)
            with nc.named_scope("scaleA"):
                # per-tile DVE abs-free reduce (host pre-abs'd wA); the
                # serial reduce chain (~72us) paces just behind the 3-queue
                # DMA stream (~68us at 4KB lines). (An in-place
                # tensor_tensor accumulator would run at ~1 elem/cyc vs
                # 0.65 but hangs the device; a PE ones-matmul fold is
                # faster still but inflates the PE-triggered collectives
                # init barrier 50->115us, which gates the AllReduce.)
                partials = small.tile([128, NWA], F32)
                WQ_W = (85.0, 72.0, 64.0)
                counts = [0, 0, 0]
                for i in range(NWA):
                    qsel = min(range(3), key=lambda q: (counts[q] + 1) / WQ_W[q])
                    counts[qsel] += 1
                    wt = wf.tile([128, 2048], BF16, tag="w", name=f"wa_{i}")
                    QUEUES[qsel].dma_start(
                        wt[:], wA[i * 128:(i + 1) * 128, :])
                    nc.vector.tensor_reduce(
                        partials[:, i:i + 1], wt[:], AX.X, ALU.add)
                col = small.tile([128, 1], F32)
                nc.vector.tensor_reduce(col[:], partials[:], AX.X, ALU.add)
                ones = small.tile([128, 1], F32)
                nc.vector.memset(ones[:], 1.0)
                ps_scalar = psum.tile([1, 1], F32, tag="acc")
                nc.tensor.matmul(ps_scalar[:], ones[:], col[:])
                local_sum = small.tile([1, 1], F32)
                nc.vector.tensor_copy(local_sum[:], ps_scalar[:])

                in_b = dram.tile([1, 1], F32)
                out_b = dram.tile([1, 1], F32)
                nc.gpsimd.dma_start(in_b[:], local_sum[:])
                nc.gpsimd.collective_compute(
                    "AllReduce", ALU.add,
                    replica_groups=[list(range(N_CORES))],
                    ins=[in_b[:]], outs=[out_b[:]])
                gsum = small.tile([1, 1], F32)
                nc.gpsimd.dma_start(gsum[:], out_b[:])

            # thr = (gsum * 2^-26) * 0.7 ; matches reference rounding
            thr1 = small.tile([1, 1], F32)
            nc.vector.tensor_scalar(thr1[:], gsum[:], INV_N, 0.7,
                                    ALU.mult, ALU.mult)
            thr = small.tile([128, 1], F32)
            nc.gpsimd.partition_broadcast(thr[:], thr1[:])
            nthr = small.tile([128, 1], F32)
            nc.vector.tensor_scalar_mul(nthr[:], thr[:], -1.0)

            if with_bias:
                bias_sb = small.tile([128, O], F32)
                nc.gpsimd.dma_start(bias_sb[:],
                                    bias.ap().to_broadcast((128, O)))

            # x pack prefetch: strip tiles t0..3 plus t4,t5, behind wA.
            xpk_r = xpk.ap().rearrange(
                "(tt p) (j two c) -> p tt j two c", p=128, two=2, c=128)
            x_tiles = {}
            for t in range(STRIP + 1):
                x_sb = xp.tile([128, NJ, 2, 128], F8, tag="x",
                               name=f"x_{t}")
                (nc.scalar if t % 2 else nc.sync).dma_start(
                    x_sb[:], xpk_r[:, t])
                x_tiles[t] = x_sb

            # wq[(kp, qc)]: [128, 2slot, 1024o] fp8 DoubleRow moving tiles
            wq = {}
            for qc in range(N_QC):
                for kp in range(KP):
                    wq[(kp, qc)] = wqp.tile([128, 2, 1024], F8, tag="wq",
                                            name=f"wq_{qc}_{kp}")

            wTq_r = [wTq[qc].ap().rearrange(
                "(kp two p) c -> kp p two c", p=128, two=2)
                for qc in range(N_QC)]
            PAIRS = [(qc, kp) for qc in range(N_QC) for kp in range(KP)]
            wb_tiles = {}

            def wpair_trigger(g):
                qc, kp = PAIRS[g]
                wt = wb.tile([128, 2, 1024], F32, tag="wb",
                             name=f"wb_{qc}_{kp}")
                QUEUES[g % 3].dma_start(wt[:], wTq_r[qc][kp])
                wb_tiles[(qc, kp)] = wt

            def quant_pair(g):
                """wq = (w >= thr) - (w <= -thr), f32 compares, both ops on
                DVE. All compute through flat 2D views: 3D [128,2,1024] APs
                run ~25x slower (31.9us/op); GpSimd is ~25x slower than DVE
                for this op even in 2D, so no engine split."""
                qc, kp = PAIRS[g]
                wt2 = wb_tiles[(qc, kp)][:].rearrange(
                    "p two c -> p (two c)")
                mneg = mf.tile([128, 2048], BF16, tag="mneg")
                nc.vector.tensor_scalar(
                    mneg[:], wt2, nthr[:], None, ALU.is_le)
                wq2 = wq[(kp, qc)][:].rearrange("p two c -> p (two c)")
                nc.vector.scalar_tensor_tensor(
                    wq2, wt2, thr[:], mneg[:],
                    ALU.is_ge, ALU.subtract)

            # Chain issue order: lo j (KP+kp) right after its hi j (kp).
            J_ORDER = []
            for kp in range(KP):
                J_ORDER.append(kp)
                if kp < N_LO:
                    J_ORDER.append(KP + kp)
            J_AFTER_PAIR = {kp: ([kp] + ([KP + kp] if kp < N_LO else []))
                            for kp in range(KP)}

            def mm(acc, t, j, oc):
                qc, h = divmod(oc, 2)
                kp = j if j < KP else j - KP
                nc.tensor.matmul(
                    acc[:], x_tiles[t][:, j],
                    wq[(kp, qc)][:, :, h * 512:(h + 1) * 512],
                    start=(j == J_ORDER[0]), stop=(j == J_ORDER[-1]),
                    perf_mode=DR)

            def epilogue(acc, t, oc, ep_engine, dma_eng):
                out_sb = op.tile([128, 512], BF16, tag="out",
                                 name=f"o_{t}_{oc}")
                if with_bias:
                    nc.vector.tensor_tensor(
                        out_sb[:], acc[:],
                        bias_sb[:, oc * 512:(oc + 1) * 512], ALU.add)
                elif ep_engine == 0:
                    nc.vector.tensor_copy(out_sb[:], acc[:])
                else:
                    nc.scalar.copy(out_sb[:], acc[:])
                dma_eng.dma_start(
                    y[t * 128:(t + 1) * 128, oc * 512:(oc + 1) * 512],
                    out_sb[:])

            # ---------------- phases B+C: quant + matmul -----------------
            with nc.named_scope("matmulC"):
                # Strip: per quant chunk qc, run the first STRIP token
                # tiles k-pair-synchronously with the quant stream (8 MMs
                # per hi j, +8 per lo j, all 8 PSUM banks). Emission of
                # triggers/quant/MMs is a single linear schedule so program
                # order matches the runtime pacing.
                strip_accs = {}

                def strip_open(qc):
                    for t in range(STRIP):
                        for h in range(2):
                            strip_accs[(qc, t, h)] = psum.tile(
                                [128, 512], F32, tag="acc",
                                name=f"sacc_{qc}_{t}_{h}")

                def strip_epis(qc):
                    # t0/t1 banks on vector (free first, feed the next
                    # chunk's first chains), t2/t3 on scalar (drains
                    # concurrently; at most one wb-gated wT trigger sits
                    # ahead, ~1 pair-interval).
                    for t in range(STRIP):
                        for h in range(2):
                            ep_eng = 0 if (t < 2 or with_bias) else 1
                            epilogue(strip_accs[(qc, t, h)], t,
                                     qc * 2 + h, ep_eng, nc.gpsimd)

                # Boundary plan (qc0->qc1 and qc1->steady): epilogues
                # wait on the closing chunk's last MM, and the next chunk's
                # MMs wait on the banks those epilogues free. Epilogues are
                # split 4/4 across vector and scalar so all 8 banks free in
                # ~2.8us of 2-engine drain; the first bank (t0,h0) frees
                # ~0.7us after the last MM, and the next chunk's j0+lo
                # consumption (4.2us) covers the rest. Keeping the PE gap
                # under ~3.4us also avoids the HAM MID re-throttle (a
                # bigger gap costs an extra ~7us 1.2GHz cold window).
                strip_open(0)
                for step in range(32 + QLEAD):
                    if step < 32:
                        wpair_trigger(step)
                    g = step - QLEAD
                    if g < 0:
                        continue
                    qc, kp = PAIRS[g]
                    quant_pair(g)
                    if qc == 1 and kp == 0:
                        strip_epis(0)
                        strip_open(1)
                    for j in J_AFTER_PAIR[kp]:
                        for t in range(STRIP):
                            for h in range(2):
                                mm(strip_accs[(qc, t, h)], t, j,
                                   qc * 2 + h)
                strip_epis(1)

                # steady state: token-major, oc-major chains (epilogue as
                # soon as each 512-out chunk closes -> early PSUM free,
                # short drain tail on the last tile).
                for t in range(STRIP, NT):
                    if t not in x_tiles:
                        x_sb = xp.tile([128, NJ, 2, 128], F8, tag="x",
                                       name=f"x_{t}")
                        nc.sync.dma_start(x_sb[:], xpk_r[:, t])
                        x_tiles[t] = x_sb
                    last = t == NT - 1
                    for oc in range(N_OC):
                        acc = psum.tile([128, 512], F32, tag="acc",
                                        name=f"acc_{t}_{oc}")
                        for j in J_ORDER:
                            mm(acc, t, j, oc)
                        epilogue(acc, t, oc, oc % 2,
                                 QUEUES[oc % 3] if last else nc.gpsimd)

    nc.compile()
    return nc


def get_nc(with_bias: bool):
    if with_bias not in _NC_CACHE:
        _NC_CACHE[with_bias] = build_nc(with_bias)
    return _NC_CACHE[with_bias]


def prep_in_maps(x: np.ndarray, weight: np.ndarray, bias: np.ndarray):
    """Host-side sharding/layout: fp8 hi/lo split + DoubleRow packing of
    x (replicated), shard weight/bias along out_features."""
    xt = np.ascontiguousarray(x.reshape(T, K)).astype(np.float32)
    xhi8 = xt.astype(E4NP)
    xlo8 = (xt - xhi8.astype(np.float32)).astype(E4NP)

    def pack(a8, njp):  # [T, K] fp8 -> [T=tt*128, njp*256] DoubleRow pack
        a = a8.reshape(NT, 128, KP, 2, 128)       # tt, tc, kp, two, p
        a = a[:, :, :njp]
        return a.transpose(0, 4, 2, 3, 1)         # tt, p, kp, two, tc

    xpk = np.concatenate(
        [pack(xhi8, KP).reshape(NT, 128, KP * 256),
         pack(xlo8, N_LO).reshape(NT, 128, N_LO * 256)],
        axis=2).reshape(T, XF)
    xpk = np.ascontiguousarray(xpk)

    wT_full = weight.T  # [K, O_TOTAL] view
    in_maps = []
    for c in range(N_CORES):
        w_shard = np.ascontiguousarray(wT_full[:, c * O:(c + 1) * O])
        in_maps.append({
            "xpk": xpk,
            "wT0": np.ascontiguousarray(w_shard[:, :1024]),
            "wT1": np.ascontiguousarray(w_shard[:, 1024:]),
            "wA": np.abs(w_shard).astype(ml_dtypes.bfloat16).reshape(
                NWA * 128, 2048),
            "bias": np.ascontiguousarray(
                bias[c * O:(c + 1) * O].reshape(1, O)).astype(np.float32),
        })
    return in_maps


def run_shards(in_maps, trace=False, with_bias=None):
    if with_bias is None:
        with_bias = any(np.any(m["bias"]) for m in in_maps)
    nc = get_nc(with_bias)
    return bass_utils.run_bass_kernel_spmd(
        nc, in_maps, core_ids=list(range(N_CORES)), trace=trace)


def kernel(x: np.ndarray, weight: np.ndarray, bias: np.ndarray) -> np.ndarray:
    x = np.asarray(x, dtype=np.float32)
    weight = np.asarray(weight, dtype=np.float32)
    bias = np.asarray(bias, dtype=np.float32)
    res = run_shards(prep_in_maps(x, weight, bias))
    y = np.concatenate(
        [res.results[c]["y"].astype(np.float32) for c in range(N_CORES)],
        axis=1)
    return y.reshape(B, S, O_TOTAL)


# revision 13
# speedup vs baseline: 1.0160x; 1.0160x over previous
"""BitLinear-1.58 (absmean ternary quant + linear) on 8 TRN2 NeuronCores.

Problem: x[4, 2048, 4096] f32, weight[16384, 4096] f32, bias[16384] f32.
    w_q = sign(w) * (|w| >= 0.7 * mean(|w|))   (global mean over all of w)
    y   = x @ w_q.T + bias                      -> [4, 2048, 16384] f32

Sharding (column/tensor parallel): weight & bias sharded along
out_features across 8 cores (2048 each); x replicated. Each core
computes y_shard [8192, 2048]; the host concatenates shards.

Matmul runs in fp8e4 with perf_mode=DoubleRow (256-contraction MMs).
Ternary w_q is exact in fp8. x is split on the host into x_hi = fp8(x)
and x_lo = fp8(x - x_hi); the chain contracts x_hi over all 16 k-pairs
plus x_lo over the first N_LO=8 pairs (error compensation, rel-err
1.89e-2 vs the 2e-2 gate). 24 MMs per chain, 6144 total; the PE runs
them back-to-back at ~263ns (the firmware clamps the PE to 13/16 =
1.95GHz whenever it is active, so 512 cols / 1.95GHz is the floor).

v2 schedule (from perfetto/NTFF analysis of the v1 run @1807.8us):
  - Phase A (global absmean) was the head critical path: first MM at
    178us. wA (bf16 |w|-sum input, reshaped [8192,1024] on host) now
    has absolute priority on all 3 DMA queues, weighted-striped
    (sync:scalar:gpsimd ~ 85:72:64 GB/s) so arrival order matches the
    DVE reduce FIFO and all queues drain together; wf pool deep enough
    to keep every queue pipelined. The scalar-AllReduce plumbing sits
    right behind gpsimd's (smallest) wA share so the cc trigger fires
    as soon as the local sum lands (v1 paid 11.5us trigger delay).
  - Quant was DVE-bound at 4.0us/pair (129us span) and starved the PE
    in the strip tails (16us of gaps + a 6.8us HAM cold window). Now:
    wT arrives as [128,2slot,1024] f32 pair tiles (one DMA), the
    (w <= -thr) compare runs on GpSimd, the (w>=thr)-mneg combine on
    DVE -> ~2.6us/pair, and wT pair triggers are emitted interleaved
    at a 5-pair lead so gpsimd's own wb-gated triggers can never sit
    ahead of the quant ops that free wb slots (deadlock rule).
  - Steady state is oc-major (one 24-MM chain per 512-out chunk, then
    its epilogue) so PSUM banks free early and the last tile's y
    drains overlapped -> shorter tail (v1: 16.5us after last MM).
x is fed pre-packed (host): per token tile a [128, NJ, 2, 128] fp8
block, hi k-pairs 0..15 then lo 0..7; each tile one contiguous DMA.
"""

import numpy as np
import ml_dtypes

import concourse.bacc as bacc
import concourse.mybir as mybir
import concourse.tile as tile
import concourse.bass_utils as bass_utils

F32 = mybir.dt.float32
BF16 = mybir.dt.bfloat16
F8 = mybir.dt.float8e4
DR = mybir.MatmulPerfMode.DoubleRow
ALU = mybir.AluOpType
AX = mybir.AxisListType
E4NP = ml_dtypes.float8_e4m3

N_CORES = 8
B, S, K, O_TOTAL = 4, 2048, 4096, 16384
T = B * S                  # 8192 tokens
O = O_TOTAL // N_CORES     # 2048 out features per core
KT = K // 128              # 32 k-tiles
KP = KT // 2               # 16 k-pairs (256-contraction DoubleRow MMs)
N_LO = 8                   # k-pairs with x_lo error compensation
NJ = KP + N_LO             # 24 MMs per accumulation chain
XF = NJ * 256              # fp8 bytes/partition per token tile
N_OC = O // 512            # 4 output chunks of 512
N_QC = O // 1024           # 2 quant chunks of 1024
NT = T // 128              # 64 token tiles
STRIP = 4                  # leading token tiles, k-synchronous with quant
NWA = 32                   # wA tiles [128, 2048] bf16 (4KB lines)
QLEAD = 5                  # wT pair-trigger lead over quant (< wb bufs)
INV_N = 1.0 / (O_TOTAL * K)  # 2^-26, exact power of two

_NC_CACHE = {}


def build_nc(with_bias: bool):
    nc = bacc.Bacc("TRN2", target_bir_lowering=False, debug=False,
                   num_devices=N_CORES)
    xpk = nc.dram_tensor("xpk", [T, XF], F8, kind="ExternalInput")
    # w^T f32 split by quant chunk; pair kp of chunk qc = rows
    # 256kp..256kp+255 -> one [128, 2, 1024] slot-major DMA.
    wTq = [nc.dram_tensor(f"wT{qc}", [K, 1024], F32, kind="ExternalInput")
           for qc in range(N_QC)]
    wA = nc.dram_tensor("wA", [NWA * 128, 2048], BF16, kind="ExternalInput")
    bias = nc.dram_tensor("bias", [1, O], F32, kind="ExternalInput")
    y = nc.dram_tensor("y", [T, O], BF16, kind="ExternalOutput")

    with tile.TileContext(nc) as tc:
        with (
            tc.tile_pool(name="wf", bufs=5) as wf,
            tc.tile_pool(name="wb", bufs=6) as wb,        # f32 pair tiles
            tc.tile_pool(name="mf", bufs=3) as mf,        # quant mask pairs
            tc.tile_pool(name="wqp", bufs=KP * N_QC) as wqp,  # ternary w
            tc.tile_pool(name="xp", bufs=STRIP + 1) as xp,  # x pack staging
            tc.tile_pool(name="op", bufs=12) as op,
            tc.tile_pool(name="small", bufs=1) as small,
            tc.tile_pool(name="psum", bufs=8, space="PSUM") as psum,
            tc.tile_pool(name="dram", bufs=1, space="DRAM") as dram,
        ):
            QUEUES = [nc.sync, nc.scalar, nc.gpsimd]
            # DMA queue plan (only sync/scalar/gpsimd can start DMAs; order
            # per queue == program order):
            #   all three: wA first (weighted-striped), nothing ahead of it.
            #   sync:   wA share -> x strip t0,t2 + t4,t5 -> wT pairs g%3==0
            #           -> x steady t6..
            #   scalar: wA share -> x strip t1,t3 -> wT pairs g%3==1 ->
            #           strip y leftovers / steady epilogue copies (ACT)
            #   gpsimd: wA share (smallest) -> allreduce plumbing -> thr
            #           broadcast -> [wT pairs g%3==2 interleaved with mneg
            #           ops at QLEAD] -> y outs
            # Deadlock rule: a wb-gated wT trigger on gpsimd at pair g sits
            # after mneg(g-QLEAD); slot for g frees when pair g-6 is
            # consumed, whose mneg is at g-1 < g in gpsimd FIFO. OK.

            # ---------------- phase A: global absmean threshold ----------
            # |w| (abs applied on host, bf16, 4KB lines) streamed at full
            # 3-queue rate, DVE-reduced per tile. (A PE ones-matmul fold is
            # faster for local_sum but inflates the collectives init
            # barrier 50->115us - the barrier is PE-triggered - so the cc
            # chain, which the AllReduce queues behind, ends up later.)
            with nc.named_scope("scaleA"):
                # ping-pong DVE accumulate: acc[i%2] = acc[(i-1)%2] + |w|
                # tile (host pre-abs'd). tensor_tensor runs ~1 elem/cyc vs
                # tensor_reduce's 0.65, so the serial chain (~48us) hides
                # behind the 3-queue DMA stream (~68us at 4KB lines). Not
                # in-place: out aliasing in0 hangs the device. (A PE
                # ones-matmul fold is faster still but inflates the
                # PE-triggered collectives init barrier 50->115us, which
                # gates the AllReduce.)
                accs = [small.tile([128, 2048], F32, name="accA"),
                        small.tile([128, 2048], F32, name="accB")]
                WQ_W = (85.0, 72.0, 64.0)
                counts = [0, 0, 0]
                for i in range(NWA):
                    qsel = min(range(3), key=lambda q: (counts[q] + 1) / WQ_W[q])
                    counts[qsel] += 1
                    wt = wf.tile([128, 2048], BF16, tag="w", name=f"wa_{i}")
                    QUEUES[qsel].dma_start(
                        wt[:], wA[i * 128:(i + 1) * 128, :])
                    if i == 0:
                        nc.vector.tensor_copy(accs[0][:], wt[:])
                    else:
                        nc.vector.tensor_tensor(
                            accs[i % 2][:], accs[(i - 1) % 2][:], wt[:],
                            ALU.add)
                col = small.tile([128, 1], F32)
                nc.vector.tensor_reduce(
                    col[:], accs[(NWA - 1) % 2][:], AX.X, ALU.add)
                ones = small.tile([128, 1], F32)
                nc.vector.memset(ones[:], 1.0)
                ps_scalar = psum.tile([1, 1], F32, tag="acc")
                nc.tensor.matmul(ps_scalar[:], ones[:], col[:])
                local_sum = small.tile([1, 1], F32)
                nc.vector.tensor_copy(local_sum[:], ps_scalar[:])

                in_b = dram.tile([1, 1], F32)
                out_b = dram.tile([1, 1], F32)
                nc.gpsimd.dma_start(in_b[:], local_sum[:])
                nc.gpsimd.collective_compute(
                    "AllReduce", ALU.add,
                    replica_groups=[list(range(N_CORES))],
                    ins=[in_b[:]], outs=[out_b[:]])
                # gsum lands broadcast across partitions via a 0-stride
                # DRAM read (partition_broadcast is a gpsimd custom op
                # costing ~6.3us incl. a library swap).
                gsumb = small.tile([128, 1], F32)
                nc.gpsimd.dma_start(gsumb[:],
                                    out_b[:].to_broadcast((128, 1)))

            # thr = (gsum * 2^-26) * 0.7 ; matches reference rounding
            thr = small.tile([128, 1], F32)
            nc.vector.tensor_scalar(thr[:], gsumb[:], INV_N, 0.7,
                                    ALU.mult, ALU.mult)
            nthr = small.tile([128, 1], F32)
            nc.vector.tensor_scalar_mul(nthr[:], thr[:], -1.0)

            if with_bias:
                bias_sb = small.tile([128, O], F32)
                nc.gpsimd.dma_start(bias_sb[:],
                                    bias.ap().to_broadcast((128, O)))

            # x pack prefetch: strip tiles t0..3 plus t4,t5, behind wA.
            xpk_r = xpk.ap().rearrange(
                "(tt p) (j two c) -> p tt j two c", p=128, two=2, c=128)
            x_tiles = {}
            for t in range(STRIP + 1):
                x_sb = xp.tile([128, NJ, 2, 128], F8, tag="x",
                               name=f"x_{t}")
                (nc.scalar if t % 2 else nc.sync).dma_start(
                    x_sb[:], xpk_r[:, t])
                x_tiles[t] = x_sb

            # wq[(kp, qc)]: [128, 2slot, 1024o] fp8 DoubleRow moving tiles
            wq = {}
            for qc in range(N_QC):
                for kp in range(KP):
                    wq[(kp, qc)] = wqp.tile([128, 2, 1024], F8, tag="wq",
                                            name=f"wq_{qc}_{kp}")

            wTq_r = [wTq[qc].ap().rearrange(
                "(kp two p) c -> kp p two c", p=128, two=2)
                for qc in range(N_QC)]
            PAIRS = [(qc, kp) for qc in range(N_QC) for kp in range(KP)]
            wb_tiles = {}

            def wpair_trigger(g):
                qc, kp = PAIRS[g]
                wt = wb.tile([128, 2, 1024], F32, tag="wb",
                             name=f"wb_{qc}_{kp}")
                QUEUES[g % 3].dma_start(wt[:], wTq_r[qc][kp])
                wb_tiles[(qc, kp)] = wt

            def quant_pair(g):
                """wq = (w >= thr) - (w <= -thr), f32 compares, both ops on
                DVE. All compute through flat 2D views: 3D [128,2,1024] APs
                run ~25x slower (31.9us/op); GpSimd is ~25x slower than DVE
                for this op even in 2D, so no engine split."""
                qc, kp = PAIRS[g]
                wt2 = wb_tiles[(qc, kp)][:].rearrange(
                    "p two c -> p (two c)")
                mneg = mf.tile([128, 2048], BF16, tag="mneg")
                nc.vector.tensor_scalar(
                    mneg[:], wt2, nthr[:], None, ALU.is_le)
                wq2 = wq[(kp, qc)][:].rearrange("p two c -> p (two c)")
                nc.vector.scalar_tensor_tensor(
                    wq2, wt2, thr[:], mneg[:],
                    ALU.is_ge, ALU.subtract)

            # Chain issue order: lo j (KP+kp) right after its hi j (kp).
            J_ORDER = []
            for kp in range(KP):
                J_ORDER.append(kp)
                if kp < N_LO:
                    J_ORDER.append(KP + kp)
            J_AFTER_PAIR = {kp: ([kp] + ([KP + kp] if kp < N_LO else []))
                            for kp in range(KP)}

            def mm(acc, t, j, oc):
                qc, h = divmod(oc, 2)
                kp = j if j < KP else j - KP
                nc.tensor.matmul(
                    acc[:], x_tiles[t][:, j],
                    wq[(kp, qc)][:, :, h * 512:(h + 1) * 512],
                    start=(j == J_ORDER[0]), stop=(j == J_ORDER[-1]),
                    perf_mode=DR)

            def epilogue(acc, t, oc, ep_engine, dma_eng):
                out_sb = op.tile([128, 512], BF16, tag="out",
                                 name=f"o_{t}_{oc}")
                if with_bias:
                    nc.vector.tensor_tensor(
                        out_sb[:], acc[:],
                        bias_sb[:, oc * 512:(oc + 1) * 512], ALU.add)
                elif ep_engine == 0:
                    nc.vector.tensor_copy(out_sb[:], acc[:])
                else:
                    nc.scalar.copy(out_sb[:], acc[:])
                dma_eng.dma_start(
                    y[t * 128:(t + 1) * 128, oc * 512:(oc + 1) * 512],
                    out_sb[:])

            # ---------------- phases B+C: quant + matmul -----------------
            with nc.named_scope("matmulC"):
                # Strip: per quant chunk qc, run the first STRIP token
                # tiles k-pair-synchronously with the quant stream (8 MMs
                # per hi j, +8 per lo j, all 8 PSUM banks). Emission of
                # triggers/quant/MMs is a single linear schedule so program
                # order matches the runtime pacing.
                strip_accs = {}

                def strip_open(qc):
                    for t in range(STRIP):
                        for h in range(2):
                            strip_accs[(qc, t, h)] = psum.tile(
                                [128, 512], F32, tag="acc",
                                name=f"sacc_{qc}_{t}_{h}")

                def strip_epis(qc):
                    # t0/t1 banks on vector (free first, feed the next
                    # chunk's first chains), t2/t3 on scalar (drains
                    # concurrently; at most one wb-gated wT trigger sits
                    # ahead, ~1 pair-interval).
                    for t in range(STRIP):
                        for h in range(2):
                            ep_eng = 0 if (t < 2 or with_bias) else 1
                            epilogue(strip_accs[(qc, t, h)], t,
                                     qc * 2 + h, ep_eng, nc.gpsimd)

                # Boundary plan (qc0->qc1 and qc1->steady): epilogues
                # wait on the closing chunk's last MM, and the next chunk's
                # MMs wait on the banks those epilogues free. Epilogues are
                # split 4/4 across vector and scalar so all 8 banks free in
                # ~2.8us of 2-engine drain; the first bank (t0,h0) frees
                # ~0.7us after the last MM, and the next chunk's j0+lo
                # consumption (4.2us) covers the rest. Keeping the PE gap
                # under ~3.4us also avoids the HAM MID re-throttle (a
                # bigger gap costs an extra ~7us 1.2GHz cold window).
                strip_open(0)
                for step in range(32 + QLEAD):
                    if step < 32:
                        wpair_trigger(step)
                    g = step - QLEAD
                    if g < 0:
                        continue
                    qc, kp = PAIRS[g]
                    quant_pair(g)
                    if qc == 1 and kp == 0:
                        strip_epis(0)
                        strip_open(1)
                    for j in J_AFTER_PAIR[kp]:
                        for t in range(STRIP):
                            for h in range(2):
                                mm(strip_accs[(qc, t, h)], t, j,
                                   qc * 2 + h)
                strip_epis(1)

                # steady state: token-major, oc-major chains (epilogue as
                # soon as each 512-out chunk closes -> early PSUM free,
                # short drain tail on the last tile).
                for t in range(STRIP, NT):
                    if t not in x_tiles:
                        x_sb = xp.tile([128, NJ, 2, 128], F8, tag="x",
                                       name=f"x_{t}")
                        nc.sync.dma_start(x_sb[:], xpk_r[:, t])
                        x_tiles[t] = x_sb
                    last = t == NT - 1
                    for oc in range(N_OC):
                        acc = psum.tile([128, 512], F32, tag="acc",
                                        name=f"acc_{t}_{oc}")
                        for j in J_ORDER:
                            mm(acc, t, j, oc)
                        epilogue(acc, t, oc, oc % 2,
                                 QUEUES[oc % 3] if last else nc.gpsimd)

    nc.compile()
    return nc


def get_nc(with_bias: bool):
    if with_bias not in _NC_CACHE:
        _NC_CACHE[with_bias] = build_nc(with_bias)
    return _NC_CACHE[with_bias]


def prep_in_maps(x: np.ndarray, weight: np.ndarray, bias: np.ndarray):
    """Host-side sharding/layout: fp8 hi/lo split + DoubleRow packing of
    x (replicated), shard weight/bias along out_features."""
    xt = np.ascontiguousarray(x.reshape(T, K)).astype(np.float32)
    xhi8 = xt.astype(E4NP)
    xlo8 = (xt - xhi8.astype(np.float32)).astype(E4NP)

    def pack(a8, njp):  # [T, K] fp8 -> [T=tt*128, njp*256] DoubleRow pack
        a = a8.reshape(NT, 128, KP, 2, 128)       # tt, tc, kp, two, p
        a = a[:, :, :njp]
        return a.transpose(0, 4, 2, 3, 1)         # tt, p, kp, two, tc

    xpk = np.concatenate(
        [pack(xhi8, KP).reshape(NT, 128, KP * 256),
         pack(xlo8, N_LO).reshape(NT, 128, N_LO * 256)],
        axis=2).reshape(T, XF)
    xpk = np.ascontiguousarray(xpk)

    wT_full = weight.T  # [K, O_TOTAL] view
    in_maps = []
    for c in range(N_CORES):
        w_shard = np.ascontiguousarray(wT_full[:, c * O:(c + 1) * O])
        in_maps.append({
            "xpk": xpk,
            "wT0": np.ascontiguousarray(w_shard[:, :1024]),
            "wT1": np.ascontiguousarray(w_shard[:, 1024:]),
            "wA": np.abs(w_shard).astype(ml_dtypes.bfloat16).reshape(
                NWA * 128, 2048),
            "bias": np.ascontiguousarray(
                bias[c * O:(c + 1) * O].reshape(1, O)).astype(np.float32),
        })
    return in_maps


def run_shards(in_maps, trace=False, with_bias=None):
    if with_bias is None:
        with_bias = any(np.any(m["bias"]) for m in in_maps)
    nc = get_nc(with_bias)
    return bass_utils.run_bass_kernel_spmd(
        nc, in_maps, core_ids=list(range(N_CORES)), trace=trace)


def kernel(x: np.ndarray, weight: np.ndarray, bias: np.ndarray) -> np.ndarray:
    x = np.asarray(x, dtype=np.float32)
    weight = np.asarray(weight, dtype=np.float32)
    bias = np.asarray(bias, dtype=np.float32)
    res = run_shards(prep_in_maps(x, weight, bias))
    y = np.concatenate(
        [res.results[c]["y"].astype(np.float32) for c in range(N_CORES)],
        axis=1)
    return y.reshape(B, S, O_TOTAL)
